# revision 61
# baseline (speedup 1.0000x reference)
"""TRN2 Bass kernel for nn_LocalSelfAttn (LN -> packed QKV -> banded attention
(window +-16) -> out-proj -> residual), sharded 8-way over (B, T):
8 cores x 1024 tokens, 16-token halo strips (zero-padded at sequence edges).
~78us vs the 122us baseline. Key trace-driven decisions: critical x strip
DMAs first (DMA engines round-robin ALL live transfers, so bulk constants
are deferred to the sync queue after the ht transposes); O transposed on
the PE via is_transpose matmuls (XBAR-transpose DMAs wait on SWDGE queue
counters and serialize behind output stores with no data dependency); the
band mask accumulated into scores PSUM as a -3e6 penalty via one matmul
per PSUM bank (exp underflows masked cells; no DVE mask multiply); V-slab
zeros/ones template host-baked and DMA'd (gpsimd memsets took 10us).

Design (vs the 122us baseline):
  - x strip loaded in 3 batched DMAs (HWDGE descriptor-gen is a serial
    ~630ns/instr resource; the old 10 per-tile DMAs trickled in over 16us)
  - PE p-state warmup: a chain of tiny matmuls during LN keeps the PE
    continuously busy so QKV runs at 2.4GHz instead of 1.2GHz
  - QKV projection in fp8e4m3 with perf_mode=DoubleRow (256-deep
    contraction, 0.5 cyc/row): weights prescaled x64 on host, descale
    folded into the exp() scale and the out-proj epilogue
  - ht transposed via 3 batched XBAR DMAs (bf16), then cast to fp8 flat
    layout [d_chunk, tok] so matmul operands can slice arbitrary token
    windows
  - attention in 11 blocks of 96 queries x 128-key windows: no hi/lo
    split matmuls; one exp + one mask-mul per block
  - PV via fp8 DoubleRow pairing two heads per matmul (zero-padded V
    slabs); ones-columns give per-head softmax rowsums for free
  - scores pairs run even/odd head row-groups concurrently; sT PSUM is
    double-buffered (2 banks each) so next-block scores overlap exp
  - out-proj (fp8 DR) emitted as soon as its 128-query osT columns are
    transposed+cast; residual fetched separately from DRAM in f32
    (improves accuracy; off critical path on the gpsimd SWDGE queue)
"""

import sys

for _p in ("/opt/trn_rl_repo",):
    if _p not in sys.path:
        sys.path.insert(0, _p)

import numpy as np
import ml_dtypes

import concourse.bass as bass
import concourse.tile as tile
from concourse import bacc, mybir
from concourse.bass import ts
from concourse.bass_utils import run_bass_kernel_spmd

F32 = mybir.dt.float32
FP8 = mybir.dt.float8e4
BF16 = mybir.dt.bfloat16
AF = mybir.ActivationFunctionType
ALU = mybir.AluOpType
DR = mybir.MatmulPerfMode.DoubleRow

B, T, D, H, BAND = 2, 4096, 512, 8, 16
DH = D // H            # 64
LN_EPS = 1e-5
N_CORES = 8
PC = 1024              # tokens per core
HALO = 16
ST = PC + 2 * HALO     # strip = 1056 real tokens
STP = 1152             # strip padded to 9 full LN tiles
NT = 9                 # LN tiles
QB = 96                # queries per attention block
NBQ = 11               # 10 x 96 + 1 x 64
WIN = 128              # key window per block
NOB = 8                # out-proj blocks of 128 queries
W8SC = 64.0            # fp8 weight prescale
EXPSC = 1.0 / (W8SC * W8SC * np.sqrt(DH))   # descale both x64 + 1/sqrt(dh)
MNEG = -3.0e6          # masked-score additive penalty (pre-EXPSC scale)
NWARM = 40

# out-proj block ob -> first attention block b whose osT cast completes
# columns [128*ob, 128*ob+128)
OB_DEADLINE = {0: 1, 1: 2, 2: 3, 3: 5, 4: 6, 5: 7, 6: 9, 7: 10}


def w0_of(b):
    return 96 * b if b < 10 else 928


_NC_CACHE = None


def build_bass():
    nc = bacc.Bacc(None, target_bir_lowering=False)
    xin = nc.declare_dram_parameter("xin", [128, NT, D], BF16, isOutput=False)
    xres = nc.declare_dram_parameter("xres", [128, NOB, D], F32, isOutput=False)
    w8qk = nc.declare_dram_parameter("w8qk", [128, 2, 2, 2 * D], FP8, isOutput=False)
    w8v = nc.declare_dram_parameter("w8v", [128, 2, 2, D], FP8, isOutput=False)
    wo8 = nc.declare_dram_parameter("wo8", [2, 128, 2, D], FP8, isOutput=False)
    beffqk = nc.declare_dram_parameter("beffqk", [128, 8], F32, isOutput=False)
    v8init = nc.declare_dram_parameter("v8init", [128, NBQ * 4 * 2 * 132], FP8,
                                       isOutput=False)
    cmask = nc.declare_dram_parameter("cmask", [QB, NBQ, 128], BF16, isOutput=False)
    ieye = nc.declare_dram_parameter("ieye", [QB, 4 * QB], BF16, isOutput=False)
    yout = nc.declare_dram_parameter("yout", [PC, D], F32, isOutput=True)

    with tile.TileContext(nc) as tc:
        from contextlib import ExitStack

        with ExitStack() as ctx:
            const = ctx.enter_context(tc.tile_pool(name="const", bufs=1))
            sb = ctx.enter_context(tc.tile_pool(name="sb", bufs=1))
            ln = ctx.enter_context(tc.tile_pool(name="ln", bufs=8))
            cpq = ctx.enter_context(tc.tile_pool(name="cpq", bufs=2))
            at = ctx.enter_context(tc.tile_pool(name="at", bufs=3))
            osTp = ctx.enter_context(tc.tile_pool(name="osTp", bufs=3))

            # ---- x strip in 3 batched DMAs on sync queue; host pre-packs it
            # partition-major so each partition is ONE big descriptor ----
            x_sb = sb.tile([128, NT, D], BF16)
            XG = [(0, 3), (3, 6), (6, 9)]
            for (ta, tb) in XG:
                nc.sync.dma_start(out=x_sb[:, ta:tb, :], in_=xin[:, ta:tb, :])

            # ---- early constants: only what phase A needs up front ----
            beff_sb = const.tile([128, 8], F32)
            nc.scalar.dma_start(out=beff_sb, in_=beffqk[:, :])
            w8_sb = const.tile([128, 2, 2, 2 * D], FP8)
            nc.scalar.dma_start(out=w8_sb, in_=w8qk[:, :, :, :])
            w8v_sb = const.tile([128, 2, 2, D], FP8)
            nc.scalar.dma_start(out=w8v_sb, in_=w8v[:, :, :, :])
            v8 = sb.tile([128, NBQ, 4, 2, 132], FP8)   # per-block DR V slabs
            eps_sb = const.tile([128, 1], F32)
            nc.vector.memset(eps_sb, LN_EPS)

            # ---- persistent activations ----
            hbf = sb.tile([128, NT, D], BF16)          # normalized h, token-major
            htst = sb.tile([128, 36, 128], BF16)       # h^T staging (tile,dc)-major
            ht8 = sb.tile([128, 4, STP], FP8)          # h^T fp8 flat [dc, tok]
            qk_sb = sb.tile([128, 8, 1088], BF16)      # q,k rows x64, flat tok
            nc.gpsimd.memset(qk_sb[:, 0:4, 1040:1088], 0.0)  # block-10 query pad
            osT8 = sb.tile([128, 4, PC], FP8)          # O^T fp8 [dc, q]

            # ================= Phase A =================
            # PE p-state warmup: chained tiny matmuls during LN keep the PE
            # continuously busy so the first real matmuls run at full clock
            with tc.tile_pool(name="psW", bufs=1, space="PSUM") as psW:
                wt = psW.tile([128, 256], F32)
                for _ in range(NWARM):
                    nc.tensor.matmul(wt[0:2, :], lhsT=x_sb[:, 0, 0:2],
                                     rhs=x_sb[:, 0, 0:256], start=True, stop=True)

            # LayerNorm emitted group-locally (3 tiles per XBAR transpose
            # group) so transpose g0 / cast g0 fire as early as possible
            NRMS = [0, 1, 0, 1, 0, 1, 0, 1, 0]   # 0=scalar, 1=vector
            for g in range(3):
                for t in range(3 * g, 3 * g + 3):
                    stats = ln.tile([128, 6], F32)
                    nc.vector.bn_stats(out=stats, in_=x_sb[:, t, :])
                    mv = ln.tile([128, 2], F32)
                    nc.vector.bn_aggr(out=mv, in_=stats)
                    std = ln.tile([128, 1], F32)
                    nc.scalar.activation(out=std, in_=mv[:, 1:2], func=AF.Sqrt,
                                         bias=eps_sb)
                    rstd = ln.tile([128, 1], F32)
                    nc.vector.reciprocal(out=rstd, in_=std)
                    nbias = ln.tile([128, 1], F32)
                    nc.vector.tensor_scalar(
                        out=nbias, in0=mv[:, 0:1], scalar1=rstd, scalar2=-1.0,
                        op0=ALU.mult, op1=ALU.mult)
                    if NRMS[t] == 0:
                        nc.scalar.activation(out=hbf[:, t, :], in_=x_sb[:, t, :],
                                             func=AF.Identity, bias=nbias,
                                             scale=rstd)
                    else:
                        nc.vector.tensor_scalar(
                            out=hbf[:, t, :], in0=x_sb[:, t, :], scalar1=rstd,
                            scalar2=nbias, op0=ALU.mult, op1=ALU.add)
                nc.sync.dma_start_transpose(
                    htst[:, 12 * g:12 * g + 12, :], hbf[:, 3 * g:3 * g + 3, :])
                # cast to fp8 + permute (tile,dc)-major -> flat [dc, tok];
                # split 2:1 across DVE and Act to halve the latency
                # (gpsimd handles fp8 ~10x slower: keep off it)
                dst1 = ht8[:, :, 384 * g:384 * g + 256].rearrange(
                    "p c (t x) -> p t c x", x=128)
                src1 = htst[:, 12 * g:12 * g + 8, :].rearrange(
                    "p (t c) x -> p t c x", c=4)
                nc.vector.tensor_copy(out=dst1, in_=src1)
                dst2 = ht8[:, :, 384 * g + 256:384 * g + 384].rearrange(
                    "p c (t x) -> p t c x", x=128)
                src2 = htst[:, 12 * g + 8:12 * g + 12, :].rearrange(
                    "p (t c) x -> p t c x", c=4)
                nc.scalar.activation(out=dst2, in_=src2, func=AF.Identity)

            # ---- late constants on the SYNC queue, in-order AFTER the ht
            # transposes: DMA engines share bandwidth round-robin across all
            # live transfers, so issuing these early starves the critical x
            # strip loads (x was landing at 26us with these in flight)
            cm_sb = const.tile([QB, NBQ, 128], BF16)   # -3e6 * (1-mask)
            nc.sync.dma_start(out=cm_sb, in_=cmask[:, :, :])
            ie_sb = const.tile([QB, 4 * QB], BF16)     # identity tiled 4x
            nc.sync.dma_start(out=ie_sb, in_=ieye[:, :])
            nc.sync.dma_start(                          # zeros + ones prebaked
                out=v8[:, :, :, :, :], in_=v8init[:, :])
            wo_sb = const.tile([128, 2, 2, D], FP8)
            nc.sync.dma_start(out=wo_sb, in_=wo8.rearrange("c p k j -> p c k j"))
            xres_sb = const.tile([128, NOB, D], F32)
            for g in range(2):
                nc.sync.dma_start(
                    out=xres_sb[:, 4 * g:4 * g + 4, :],
                    in_=xres[:, 4 * g:4 * g + 4, :])

            with tc.tile_pool(name="psA", bufs=2, space="PSUM") as psA:
                # V projection for per-block 128-token key windows (fp8 DR).
                # Emits the DR rhs slabs [w, 2, 132] with head pairs zero-
                # padded; data cols [66k, 66k+64), ones col at 66k+64.
                def emit_vproj(b):
                    w0 = w0_of(b)
                    vp = psA.tile([128, 4, 2, 64], F32, tag="v", name="vp")
                    for c in range(2):
                        nc.tensor.matmul(
                            vp[:, :, :, :], lhsT=ht8[:, 2 * c:2 * c + 2, w0:w0 + 128],
                            rhs=w8v_sb[:, c, :, :], start=(c == 0), stop=(c == 1),
                            perf_mode=DR)
                    nc.vector.tensor_copy(
                        out=v8[:, b, :, 0, 0:64], in_=vp[:, :, 0, :])
                    nc.scalar.activation(
                        out=v8[:, b, :, 1, 66:130], in_=vp[:, :, 1, :],
                        func=AF.Identity)

                emit_vproj(0)
                emit_vproj(1)

                # Q (jc 0-3, tokens [16,1040)) and K (jc 4-7, tokens [0,1056))
                QCH = [(16, 528), (528, 1040)]
                KCH = [(0, 512), (512, 1024), (1024, 1056)]
                vnext = 2
                for jc in range(8):
                    t_lo, chunks = (16, QCH) if jc < 4 else (0, KCH)
                    qp = psA.tile([128, 1056], F32, tag="qk", name="qp")
                    for (s0, s1) in chunks:
                        for c in range(2):
                            nc.tensor.matmul(
                                qp[:, s0 - t_lo:s1 - t_lo],
                                lhsT=w8_sb[:, c, :, ts(jc, 128)],
                                rhs=ht8[:, 2 * c:2 * c + 2, s0:s1],
                                start=(c == 0), stop=(c == 1), perf_mode=DR)
                    if vnext < NBQ:
                        emit_vproj(vnext)
                        vnext += 1
                    # PSUM -> SBUF bf16 with (x64-prescaled) bias add; split
                    # in two halves so early attention blocks unblock before
                    # the full token range is evacuated
                    width = chunks[-1][1] - t_lo
                    for (h0, h1) in ((0, 512), (512, width)):
                        if jc % 2 == 0:
                            nc.scalar.activation(
                                out=qk_sb[:, jc, t_lo + h0:t_lo + h1],
                                in_=qp[:, h0:h1], func=AF.Identity,
                                bias=beff_sb[:, jc:jc + 1])
                        else:
                            nc.vector.tensor_scalar_add(
                                out=qk_sb[:, jc, t_lo + h0:t_lo + h1],
                                in0=qp[:, h0:h1], scalar1=beff_sb[:, jc:jc + 1])
                while vnext < NBQ:
                    emit_vproj(vnext)
                    vnext += 1

            # ================= Phase B: attention =================
            with (
                tc.tile_pool(name="psS", bufs=2, space="PSUM") as psS,
                tc.tile_pool(name="psO", bufs=1, space="PSUM") as psO,
                tc.tile_pool(name="psY", bufs=1, space="PSUM") as psY,
                tc.tile_pool(name="psT", bufs=1, space="PSUM") as psT,
            ):
                def emit_outproj(ob):
                    yp = psY.tile([128, D], F32, tag="yp", name="yp")
                    for c in range(2):
                        nc.tensor.matmul(
                            yp, lhsT=osT8[:, 2 * c:2 * c + 2, ts(ob, 128)],
                            rhs=wo_sb[:, c, :, :], start=(c == 0), stop=(c == 1),
                            perf_mode=DR)
                    ysb = at.tile([128, D], F32, tag="ysb", name="ysb")
                    nc.vector.scalar_tensor_tensor(
                        out=ysb, in0=yp, scalar=1.0 / W8SC,
                        in1=xres_sb[:, ob, :], op0=ALU.mult, op1=ALU.add)
                    nc.gpsimd.dma_start(out=yout[ts(ob, 128), :], in_=ysb)

                # software pipeline: block b's PV/normalize/transpose are
                # emitted during block b+1, so the in-order PE queue never
                # waits on exp(b) -- it does PV(b-1)/transposes(b-1) instead
                emit_at = {dl + 2: ob for ob, dl in OB_DEADLINE.items()
                           if dl + 2 <= 10}
                pth_ring = {}

                def emit_tail(bb):
                    nqb = QB if bb < 10 else 64
                    pth = pth_ring.pop(bb)
                    # PV: fp8 DR, two heads per matmul via zero-padded slabs
                    ova = psO.tile([128, 3, 2, 66], F32, tag="ova")
                    ovb = psO.tile([128, 1, 2, 66], F32, tag="ovb")
                    for i in range(4):
                        out = ova[0:QB, i, :, :] if i < 3 else ovb[0:QB, 0, :, :]
                        nc.tensor.matmul(
                            out, lhsT=pth[:, i, :, :], rhs=v8[:, bb, i, :, :],
                            start=True, stop=True, perf_mode=DR)
                    # normalize: rowsums at col 64 of each slab
                    rsa = at.tile([128, 3, 2, 1], F32, tag="rsa")
                    nc.vector.reciprocal(out=rsa[0:QB], in_=ova[0:QB, :, :, 64:65])
                    rsb = at.tile([128, 1, 2, 1], F32, tag="rsb")
                    nc.vector.reciprocal(out=rsb[0:QB], in_=ovb[0:QB, :, :, 64:65])
                    osq = at.tile([128, 8, DH], BF16, tag="osq")
                    nc.vector.scalar_tensor_tensor(
                        out=osq[0:QB, 0:6, :].rearrange("p (i k) d -> p i k d", k=2),
                        in0=ova[0:QB, :, :, 0:64], scalar=1.0 / W8SC,
                        in1=rsa[0:QB].to_broadcast([QB, 3, 2, 64]),
                        op0=ALU.mult, op1=ALU.mult)
                    nc.vector.scalar_tensor_tensor(
                        out=osq[0:QB, 6:8, :].rearrange("p (i k) d -> p i k d", k=2),
                        in0=ovb[0:QB, :, :, 0:64], scalar=1.0 / W8SC,
                        in1=rsb[0:QB].to_broadcast([QB, 1, 2, 64]),
                        op0=ALU.mult, op1=ALU.mult)
                    # transpose O -> [d, q] on the PE (is_transpose matmuls;
                    # avoids XBAR DMAs whose semaphores serialize against the
                    # SWDGE output stores), then cast fp8 out of PSUM
                    osT = psT.tile([128, 4, QB], BF16, tag="osT")
                    for c in range(4):
                        nc.tensor.transpose(
                            osT[:, c, 0:nqb], osq[0:nqb, 2 * c:2 * c + 2, :],
                            ie_sb[0:nqb, 0:nqb])
                    nc.scalar.activation(out=osT8[:, :, QB * bb:QB * bb + nqb],
                                         in_=osT[:, :, 0:nqb], func=AF.Identity)

                for b in range(NBQ):
                    q0 = HALO + QB * b
                    w0 = w0_of(b)
                    nq = QB if b < 10 else 64
                    # scores S^T[w, q]; head 2i+k -> slot 4k+i; even/odd head
                    # row-groups (partitions 0-63 / 64-127) run concurrently
                    # and land in different PSUM banks
                    sT = psS.tile([128, 8, 128], F32, tag="sT")
                    for i in range(4):
                        for k in range(2):
                            p64 = 64 * k
                            nc.tensor.matmul(
                                sT[:, 4 * k + i, 0:QB],
                                lhsT=qk_sb[p64:p64 + 64, 4 + i, w0:w0 + WIN],
                                rhs=qk_sb[p64:p64 + 64, i, q0:q0 + QB],
                                start=True, stop=False, skip_group_check=True)
                    # band mask applied on the PE: accumulate -3e6*(1-m) into
                    # the head slots (rhs = identity tiled 4x; one matmul per
                    # PSUM bank), so exp() underflows masked cells to 0 --
                    # no DVE mask multiply needed
                    for half in range(2):
                        nc.tensor.matmul(
                            sT[:, 4 * half:4 * half + 4, 0:QB],
                            lhsT=cm_sb[:, b, :], rhs=ie_sb[:, :],
                            start=False, stop=True, skip_group_check=True)
                    # exp (with all descales folded into scale) -> fp8 P
                    pth = at.tile([128, 4, 2, QB], FP8, tag="pth")
                    pth_ring[b] = pth
                    nc.scalar.activation(
                        out=pth[:, :, :, :].rearrange("p i k q -> p k i q"),
                        in_=sT[:, :, 0:QB].rearrange("p (k i) q -> p k i q", k=2),
                        func=AF.Exp, scale=float(EXPSC))
                    if b >= 1:
                        emit_tail(b - 1)
                    if b in emit_at:
                        emit_outproj(emit_at[b])
                # outproj(6) needs only cast(9): run it under exp(10)
                emit_outproj(6)
                emit_tail(NBQ - 1)
                emit_outproj(7)
    nc.finalize()
    return nc


def make_in_maps(x, ln_g, ln_b, w_in, b_in, w_out, b_out):
    x = np.asarray(x, np.float32)
    ln_g = np.asarray(ln_g, np.float32)
    ln_b = np.asarray(ln_b, np.float32)
    w_in = np.asarray(w_in, np.float32)
    b_in = np.asarray(b_in, np.float32)
    w_out = np.asarray(w_out, np.float32)
    b_out = np.asarray(b_out, np.float32)

    # fold LN affine into the packed projection (scores descale lives in
    # the exp scale on-device, NOT in the weights)
    w_eff = w_in * ln_g[None, :]
    b_eff = b_in + w_in @ ln_b

    bf = ml_dtypes.bfloat16
    f8 = ml_dtypes.float8_e4m3

    def dr_pack(wT, scale):     # [512, J] -> [128, 2, 2, J] fp8
        J = wT.shape[1]
        return np.ascontiguousarray(
            (wT * scale).reshape(2, 2, 128, J).transpose(2, 0, 1, 3)).astype(f8)

    w8qk = dr_pack(w_eff[:2 * D].T, W8SC)
    w8v = dr_pack(w_eff[2 * D:].T, W8SC)
    wo8 = np.ascontiguousarray(
        (w_out.T * W8SC).reshape(2, 2, 128, D).transpose(0, 2, 1, 3)).astype(f8)
    beffqk = np.ascontiguousarray(
        (b_eff[:2 * D] * W8SC).reshape(8, 128).T).astype(np.float32)
    bo_eff = b_eff[2 * D:] @ w_out.T + b_out      # v-bias folded through Wout

    # DR V slab template: zeros with rowsum ones-columns prebaked
    v8i = np.zeros((128, NBQ, 4, 2, 132), np.float32)
    v8i[:, :, :, 0, 64] = 1.0
    v8i[:, :, :, 1, 130] = 1.0
    v8i = v8i.reshape(128, -1).astype(f8)
    # identity tiled 4x: rhs of the mask-accumulate matmuls
    ie = np.ascontiguousarray(np.tile(np.eye(QB), (1, 4))).astype(bf)

    in_maps = []
    for cidx in range(N_CORES):
        batch = cidx // 4
        t0 = (cidx % 4) * PC
        xloc = np.zeros((STP, D), bf)
        lo = t0 - HALO
        s0, s1 = max(lo, 0), min(t0 + PC + HALO, T)
        xloc[s0 - lo:s1 - lo] = x[batch, s0:s1].astype(bf)
        # partition-major pack: one contiguous descriptor per partition
        xloc = np.ascontiguousarray(
            xloc.reshape(NT, 128, D).transpose(1, 0, 2))
        xr = (x[batch, t0:t0 + PC] + bo_eff[None, :]).astype(np.float32)
        xr = np.ascontiguousarray(xr.reshape(NOB, 128, D).transpose(1, 0, 2))

        # cmask[q, b, w] = MNEG * (1 - mask): band + in-batch bounds
        # (+ block-10 query padding)
        cm = np.zeros((QB, NBQ, 128), np.float32)
        ww = np.arange(128)[None, :]
        qq = np.arange(QB)[:, None]
        for b in range(NBQ):
            w0 = w0_of(b)
            keyg = t0 - HALO + w0 + ww                 # [1, 128]
            qg = t0 + QB * b + qq                      # [QB, 1]
            m = (np.abs(keyg - qg) <= BAND) & (keyg >= 0) & (keyg < T) \
                & (qg < t0 + PC)
            cm[:, b, :] = MNEG * (1.0 - m)
        in_maps.append(dict(
            xin=xloc, xres=xr, w8qk=w8qk, w8v=w8v, wo8=wo8, beffqk=beffqk,
            v8init=v8i, cmask=cm.astype(bf), ieye=ie))
    return in_maps


def kernel_run(inputs, trace=False, trace_kwargs=None):
    global _NC_CACHE
    if _NC_CACHE is None:
        _NC_CACHE = build_bass()
    nc = _NC_CACHE
    in_maps = make_in_maps(**inputs)
    kw = {}
    if trace:
        kw = dict(trace=True, trace_cores=[0], **(trace_kwargs or {}))
    res = run_bass_kernel_spmd(nc, in_maps, list(range(N_CORES)), **kw)
    y = np.stack([res.results[c]["yout"] for c in range(N_CORES)])
    out = y.reshape(B, T, D).astype(np.float32)
    return out, res


def kernel(**inputs):
    out, _ = kernel_run(inputs, trace=False)
    return out


if __name__ == "__main__":
    rng = np.random.default_rng(0)
    ins = dict(
        x=rng.standard_normal((B, T, D)).astype(np.float32),
        ln_g=np.ones(D, np.float32), ln_b=np.zeros(D, np.float32),
        w_in=(rng.standard_normal((3 * D, D)) * 0.02).astype(np.float32),
        b_in=np.zeros(3 * D, np.float32),
        w_out=(rng.standard_normal((D, D)) * 0.02).astype(np.float32),
        b_out=np.zeros(D, np.float32))
    out = kernel(**ins)
    print("ran:", out.shape, out.dtype)


# revision 62
# speedup vs baseline: 1.0123x; 1.0123x over previous
"""TRN2 Bass kernel for nn_LocalSelfAttn (LN -> packed QKV -> banded attention
(window +-16) -> out-proj -> residual), sharded 8-way over (B, T):
8 cores x 1024 tokens, 16-token halo strips (zero-padded at sequence edges).
~78us vs the 122us baseline. Key trace-driven decisions: critical x strip
DMAs first (DMA engines round-robin ALL live transfers, so bulk constants
are deferred to the sync queue after the ht transposes); O transposed on
the PE via is_transpose matmuls (XBAR-transpose DMAs wait on SWDGE queue
counters and serialize behind output stores with no data dependency); the
band mask accumulated into scores PSUM as a -3e6 penalty via one matmul
per PSUM bank (exp underflows masked cells; no DVE mask multiply); V-slab
zeros/ones template host-baked and DMA'd (gpsimd memsets took 10us).

Design (vs the 122us baseline):
  - x strip loaded in 3 batched DMAs (HWDGE descriptor-gen is a serial
    ~630ns/instr resource; the old 10 per-tile DMAs trickled in over 16us)
  - PE p-state warmup: a chain of tiny matmuls during LN keeps the PE
    continuously busy so QKV runs at 2.4GHz instead of 1.2GHz
  - QKV projection in fp8e4m3 with perf_mode=DoubleRow (256-deep
    contraction, 0.5 cyc/row): weights prescaled x64 on host, descale
    folded into the exp() scale and the out-proj epilogue
  - ht transposed via 3 batched XBAR DMAs (bf16), then cast to fp8 flat
    layout [d_chunk, tok] so matmul operands can slice arbitrary token
    windows
  - attention in 11 blocks of 96 queries x 128-key windows: no hi/lo
    split matmuls; one exp + one mask-mul per block
  - PV via fp8 DoubleRow pairing two heads per matmul (zero-padded V
    slabs); ones-columns give per-head softmax rowsums for free
  - scores pairs run even/odd head row-groups concurrently; sT PSUM is
    double-buffered (2 banks each) so next-block scores overlap exp
  - out-proj (fp8 DR) emitted as soon as its 128-query osT columns are
    transposed+cast; residual fetched separately from DRAM in f32
    (improves accuracy; off critical path on the gpsimd SWDGE queue)
"""

import sys

for _p in ("/opt/trn_rl_repo",):
    if _p not in sys.path:
        sys.path.insert(0, _p)

import numpy as np
import ml_dtypes

import concourse.bass as bass
import concourse.tile as tile
from concourse import bacc, mybir
from concourse.bass import ts
from concourse.bass_utils import run_bass_kernel_spmd

F32 = mybir.dt.float32
FP8 = mybir.dt.float8e4
BF16 = mybir.dt.bfloat16
AF = mybir.ActivationFunctionType
ALU = mybir.AluOpType
DR = mybir.MatmulPerfMode.DoubleRow

B, T, D, H, BAND = 2, 4096, 512, 8, 16
DH = D // H            # 64
LN_EPS = 1e-5
N_CORES = 8
PC = 1024              # tokens per core
HALO = 16
ST = PC + 2 * HALO     # strip = 1056 real tokens
STP = 1152             # strip padded to 9 full LN tiles
NT = 9                 # LN tiles
QB = 96                # queries per attention block
NBQ = 11               # 10 x 96 + 1 x 64
WIN = 128              # key window per block
NOB = 8                # out-proj blocks of 128 queries
W8SC = 64.0            # fp8 weight prescale
EXPSC = 1.0 / (W8SC * W8SC * np.sqrt(DH))   # descale both x64 + 1/sqrt(dh)
MNEG = -3.0e6          # masked-score additive penalty (pre-EXPSC scale)
NWARM = 40

# out-proj block ob -> first attention block b whose osT cast completes
# columns [128*ob, 128*ob+128)
OB_DEADLINE = {0: 1, 1: 2, 2: 3, 3: 5, 4: 6, 5: 7, 6: 9, 7: 10}


def w0_of(b):
    return 96 * b if b < 10 else 928


_NC_CACHE = None


def build_bass():
    nc = bacc.Bacc(None, target_bir_lowering=False)
    xin = nc.declare_dram_parameter("xin", [128, NT, D], BF16, isOutput=False)
    xres = nc.declare_dram_parameter("xres", [128, NOB, D], F32, isOutput=False)
    w8qk = nc.declare_dram_parameter("w8qk", [128, 2, 2, 2 * D], FP8, isOutput=False)
    w8v = nc.declare_dram_parameter("w8v", [128, 2, 2, D], FP8, isOutput=False)
    wo8 = nc.declare_dram_parameter("wo8", [2, 128, 2, D], FP8, isOutput=False)
    beffqk = nc.declare_dram_parameter("beffqk", [128, 8], F32, isOutput=False)
    v8init = nc.declare_dram_parameter("v8init", [128, NBQ * 4 * 2 * 132], FP8,
                                       isOutput=False)
    cmask = nc.declare_dram_parameter("cmask", [QB, NBQ, 128], BF16, isOutput=False)
    ieye = nc.declare_dram_parameter("ieye", [QB, 4 * QB], BF16, isOutput=False)
    yout = nc.declare_dram_parameter("yout", [PC, D], F32, isOutput=True)

    with tile.TileContext(nc) as tc:
        from contextlib import ExitStack

        with ExitStack() as ctx:
            const = ctx.enter_context(tc.tile_pool(name="const", bufs=1))
            sb = ctx.enter_context(tc.tile_pool(name="sb", bufs=1))
            ln = ctx.enter_context(tc.tile_pool(name="ln", bufs=4))
            cpq = ctx.enter_context(tc.tile_pool(name="cpq", bufs=2))
            at = ctx.enter_context(tc.tile_pool(name="at", bufs=3))
            osTp = ctx.enter_context(tc.tile_pool(name="osTp", bufs=3))

            # ---- x strip in 3 batched DMAs on sync queue; host pre-packs it
            # partition-major so each partition is ONE big descriptor ----
            x_sb = sb.tile([128, NT, D], BF16)
            XG = [(0, 3), (3, 6), (6, 9)]
            for (ta, tb) in XG:
                nc.sync.dma_start(out=x_sb[:, ta:tb, :], in_=xin[:, ta:tb, :])

            # ---- early constants: only what phase A needs up front ----
            beff_sb = const.tile([128, 8], F32)
            nc.scalar.dma_start(out=beff_sb, in_=beffqk[:, :])
            w8_sb = const.tile([128, 2, 2, 2 * D], FP8)
            nc.scalar.dma_start(out=w8_sb, in_=w8qk[:, :, :, :])
            w8v_sb = const.tile([128, 2, 2, D], FP8)
            nc.scalar.dma_start(out=w8v_sb, in_=w8v[:, :, :, :])
            v8 = sb.tile([128, NBQ, 4, 2, 132], FP8)   # per-block DR V slabs
            eps_sb = const.tile([128, 1], F32)
            nc.vector.memset(eps_sb, LN_EPS)

            # ---- persistent activations ----
            hbf = sb.tile([128, NT, D], BF16)          # normalized h, token-major
            htst = sb.tile([128, 36, 128], BF16)       # h^T staging (tile,dc)-major
            ht8 = sb.tile([128, 4, STP], FP8)          # h^T fp8 flat [dc, tok]
            qk_sb = sb.tile([128, 8, 1088], BF16)      # q,k rows x64, flat tok
            nc.gpsimd.memset(qk_sb[:, 0:4, 1040:1088], 0.0)  # block-10 query pad
            osT8 = sb.tile([128, 4, PC], FP8)          # O^T fp8 [dc, q]

            # ================= Phase A =================
            # PE p-state warmup: chained tiny matmuls during LN keep the PE
            # continuously busy so the first real matmuls run at full clock
            with tc.tile_pool(name="psW", bufs=1, space="PSUM") as psW:
                wt = psW.tile([128, 256], F32)
                for _ in range(NWARM):
                    nc.tensor.matmul(wt[0:2, :], lhsT=x_sb[:, 0, 0:2],
                                     rhs=x_sb[:, 0, 0:256], start=True, stop=True)

            # LayerNorm emitted group-locally (3 tiles per XBAR transpose
            # group) so transpose g0 / cast g0 fire as early as possible
            NRMS = [0, 1, 0, 1, 0, 1, 0, 1, 0]   # 0=scalar, 1=vector
            for g in range(3):
                for t in range(3 * g, 3 * g + 3):
                    stats = ln.tile([128, 6], F32)
                    nc.vector.bn_stats(out=stats, in_=x_sb[:, t, :])
                    mv = ln.tile([128, 2], F32)
                    nc.vector.bn_aggr(out=mv, in_=stats)
                    std = ln.tile([128, 1], F32)
                    nc.scalar.activation(out=std, in_=mv[:, 1:2], func=AF.Sqrt,
                                         bias=eps_sb)
                    rstd = ln.tile([128, 1], F32)
                    nc.vector.reciprocal(out=rstd, in_=std)
                    nbias = ln.tile([128, 1], F32)
                    nc.vector.tensor_scalar(
                        out=nbias, in0=mv[:, 0:1], scalar1=rstd, scalar2=-1.0,
                        op0=ALU.mult, op1=ALU.mult)
                    if NRMS[t] == 0:
                        nc.scalar.activation(out=hbf[:, t, :], in_=x_sb[:, t, :],
                                             func=AF.Identity, bias=nbias,
                                             scale=rstd)
                    else:
                        nc.vector.tensor_scalar(
                            out=hbf[:, t, :], in0=x_sb[:, t, :], scalar1=rstd,
                            scalar2=nbias, op0=ALU.mult, op1=ALU.add)
                nc.sync.dma_start_transpose(
                    htst[:, 12 * g:12 * g + 12, :], hbf[:, 3 * g:3 * g + 3, :])
                # cast to fp8 + permute (tile,dc)-major -> flat [dc, tok];
                # split 2:1 across DVE and Act to halve the latency
                # (gpsimd handles fp8 ~10x slower: keep off it)
                dst1 = ht8[:, :, 384 * g:384 * g + 256].rearrange(
                    "p c (t x) -> p t c x", x=128)
                src1 = htst[:, 12 * g:12 * g + 8, :].rearrange(
                    "p (t c) x -> p t c x", c=4)
                nc.vector.tensor_copy(out=dst1, in_=src1)
                dst2 = ht8[:, :, 384 * g + 256:384 * g + 384].rearrange(
                    "p c (t x) -> p t c x", x=128)
                src2 = htst[:, 12 * g + 8:12 * g + 12, :].rearrange(
                    "p (t c) x -> p t c x", c=4)
                nc.scalar.activation(out=dst2, in_=src2, func=AF.Identity)

            # ---- late constants on the SYNC queue, in-order AFTER the ht
            # transposes: DMA engines share bandwidth round-robin across all
            # live transfers, so issuing these early starves the critical x
            # strip loads (x was landing at 26us with these in flight)
            cm_sb = const.tile([QB, NBQ, 128], BF16)   # -3e6 * (1-mask)
            nc.sync.dma_start(out=cm_sb, in_=cmask[:, :, :])
            ie_sb = const.tile([QB, 4 * QB], BF16)     # identity tiled 4x
            nc.sync.dma_start(out=ie_sb, in_=ieye[:, :])
            nc.sync.dma_start(                          # zeros + ones prebaked
                out=v8[:, :, :, :, :], in_=v8init[:, :])
            wo_sb = const.tile([128, 2, 2, D], FP8)
            nc.sync.dma_start(out=wo_sb, in_=wo8.rearrange("c p k j -> p c k j"))
            xres_sb = const.tile([128, NOB, D], F32)
            for g in range(2):
                nc.sync.dma_start(
                    out=xres_sb[:, 4 * g:4 * g + 4, :],
                    in_=xres[:, 4 * g:4 * g + 4, :])

            with tc.tile_pool(name="psA", bufs=2, space="PSUM") as psA:
                # V projection for per-block 128-token key windows (fp8 DR).
                # Emits the DR rhs slabs [w, 2, 132] with head pairs zero-
                # padded; data cols [66k, 66k+64), ones col at 66k+64.
                def emit_vproj(b):
                    w0 = w0_of(b)
                    vp = psA.tile([128, 4, 2, 64], F32, tag="v", name="vp")
                    for c in range(2):
                        nc.tensor.matmul(
                            vp[:, :, :, :], lhsT=ht8[:, 2 * c:2 * c + 2, w0:w0 + 128],
                            rhs=w8v_sb[:, c, :, :], start=(c == 0), stop=(c == 1),
                            perf_mode=DR)
                    nc.vector.tensor_copy(
                        out=v8[:, b, :, 0, 0:64], in_=vp[:, :, 0, :])
                    nc.scalar.activation(
                        out=v8[:, b, :, 1, 66:130], in_=vp[:, :, 1, :],
                        func=AF.Identity)

                emit_vproj(0)
                emit_vproj(1)

                # Q (jc 0-3, tokens [16,1040)) and K (jc 4-7, tokens [0,1056))
                QCH = [(16, 528), (528, 1040)]
                KCH = [(0, 512), (512, 1024), (1024, 1056)]
                vnext = 2
                for jc in range(8):
                    t_lo, chunks = (16, QCH) if jc < 4 else (0, KCH)
                    qp = psA.tile([128, 1056], F32, tag="qk", name="qp")
                    for (s0, s1) in chunks:
                        for c in range(2):
                            nc.tensor.matmul(
                                qp[:, s0 - t_lo:s1 - t_lo],
                                lhsT=w8_sb[:, c, :, ts(jc, 128)],
                                rhs=ht8[:, 2 * c:2 * c + 2, s0:s1],
                                start=(c == 0), stop=(c == 1), perf_mode=DR)
                    if vnext < NBQ:
                        emit_vproj(vnext)
                        vnext += 1
                    # PSUM -> SBUF bf16 with (x64-prescaled) bias add; split
                    # in two halves so early attention blocks unblock before
                    # the full token range is evacuated
                    width = chunks[-1][1] - t_lo
                    for (h0, h1) in ((0, 512), (512, width)):
                        if jc % 2 == 0:
                            nc.scalar.activation(
                                out=qk_sb[:, jc, t_lo + h0:t_lo + h1],
                                in_=qp[:, h0:h1], func=AF.Identity,
                                bias=beff_sb[:, jc:jc + 1])
                        else:
                            nc.vector.tensor_scalar_add(
                                out=qk_sb[:, jc, t_lo + h0:t_lo + h1],
                                in0=qp[:, h0:h1], scalar1=beff_sb[:, jc:jc + 1])
                while vnext < NBQ:
                    emit_vproj(vnext)
                    vnext += 1

            # ================= Phase B: attention =================
            with (
                tc.tile_pool(name="psS", bufs=2, space="PSUM") as psS,
                tc.tile_pool(name="psO", bufs=1, space="PSUM") as psO,
                tc.tile_pool(name="psY", bufs=1, space="PSUM") as psY,
                tc.tile_pool(name="psT", bufs=1, space="PSUM") as psT,
            ):
                def emit_outproj(ob):
                    yp = psY.tile([128, D], F32, tag="yp", name="yp")
                    for c in range(2):
                        nc.tensor.matmul(
                            yp, lhsT=osT8[:, 2 * c:2 * c + 2, ts(ob, 128)],
                            rhs=wo_sb[:, c, :, :], start=(c == 0), stop=(c == 1),
                            perf_mode=DR)
                    ysb = at.tile([128, D], F32, tag="ysb", name="ysb")
                    nc.vector.scalar_tensor_tensor(
                        out=ysb, in0=yp, scalar=1.0 / W8SC,
                        in1=xres_sb[:, ob, :], op0=ALU.mult, op1=ALU.add)
                    nc.gpsimd.dma_start(out=yout[ts(ob, 128), :], in_=ysb)

                # software pipeline: block b's PV/normalize/transpose are
                # emitted during block b+1, so the in-order PE queue never
                # waits on exp(b) -- it does PV(b-1)/transposes(b-1) instead
                emit_at = {dl + 2: ob for ob, dl in OB_DEADLINE.items()
                           if dl + 2 <= 10}
                pth_ring = {}

                def emit_tail(bb):
                    nqb = QB if bb < 10 else 64
                    pth = pth_ring.pop(bb)
                    # PV: fp8 DR, two heads per matmul via zero-padded slabs
                    ova = psO.tile([128, 3, 2, 66], F32, tag="ova")
                    ovb = psO.tile([128, 1, 2, 66], F32, tag="ovb")
                    for i in range(4):
                        out = ova[0:QB, i, :, :] if i < 3 else ovb[0:QB, 0, :, :]
                        nc.tensor.matmul(
                            out, lhsT=pth[:, i, :, :], rhs=v8[:, bb, i, :, :],
                            start=True, stop=True, perf_mode=DR)
                    # normalize: rowsums at col 64 of each slab
                    rsa = at.tile([128, 3, 2, 1], F32, tag="rsa")
                    nc.vector.reciprocal(out=rsa[0:QB], in_=ova[0:QB, :, :, 64:65])
                    rsb = at.tile([128, 1, 2, 1], F32, tag="rsb")
                    nc.vector.reciprocal(out=rsb[0:QB], in_=ovb[0:QB, :, :, 64:65])
                    osq = at.tile([128, 8, DH], BF16, tag="osq")
                    nc.vector.scalar_tensor_tensor(
                        out=osq[0:QB, 0:6, :].rearrange("p (i k) d -> p i k d", k=2),
                        in0=ova[0:QB, :, :, 0:64], scalar=1.0 / W8SC,
                        in1=rsa[0:QB].to_broadcast([QB, 3, 2, 64]),
                        op0=ALU.mult, op1=ALU.mult)
                    nc.vector.scalar_tensor_tensor(
                        out=osq[0:QB, 6:8, :].rearrange("p (i k) d -> p i k d", k=2),
                        in0=ovb[0:QB, :, :, 0:64], scalar=1.0 / W8SC,
                        in1=rsb[0:QB].to_broadcast([QB, 1, 2, 64]),
                        op0=ALU.mult, op1=ALU.mult)
                    # transpose O -> [d, q] on the PE (is_transpose matmuls;
                    # avoids XBAR DMAs whose semaphores serialize against the
                    # SWDGE output stores), then cast fp8 out of PSUM
                    osT = psT.tile([128, 4, QB], BF16, tag="osT")
                    for c in range(4):
                        nc.tensor.transpose(
                            osT[:, c, 0:nqb], osq[0:nqb, 2 * c:2 * c + 2, :],
                            ie_sb[0:nqb, 0:nqb])
                    nc.scalar.activation(out=osT8[:, :, QB * bb:QB * bb + nqb],
                                         in_=osT[:, :, 0:nqb], func=AF.Identity)

                for b in range(NBQ):
                    q0 = HALO + QB * b
                    w0 = w0_of(b)
                    nq = QB if b < 10 else 64
                    # scores S^T[w, q]; head 2i+k -> slot 4k+i; even/odd head
                    # row-groups (partitions 0-63 / 64-127) run concurrently
                    # and land in different PSUM banks
                    sT = psS.tile([128, 8, 128], F32, tag="sT")
                    for i in range(4):
                        for k in range(2):
                            p64 = 64 * k
                            nc.tensor.matmul(
                                sT[:, 4 * k + i, 0:QB],
                                lhsT=qk_sb[p64:p64 + 64, 4 + i, w0:w0 + WIN],
                                rhs=qk_sb[p64:p64 + 64, i, q0:q0 + QB],
                                start=True, stop=False, skip_group_check=True)
                    # band mask applied on the PE: accumulate -3e6*(1-m) into
                    # the head slots (rhs = identity tiled 4x; one matmul per
                    # PSUM bank), so exp() underflows masked cells to 0 --
                    # no DVE mask multiply needed
                    for half in range(2):
                        nc.tensor.matmul(
                            sT[:, 4 * half:4 * half + 4, 0:QB],
                            lhsT=cm_sb[:, b, :], rhs=ie_sb[:, :],
                            start=False, stop=True, skip_group_check=True)
                    # exp (with all descales folded into scale) -> fp8 P
                    pth = at.tile([128, 4, 2, QB], FP8, tag="pth")
                    pth_ring[b] = pth
                    nc.scalar.activation(
                        out=pth[:, :, :, :].rearrange("p i k q -> p k i q"),
                        in_=sT[:, :, 0:QB].rearrange("p (k i) q -> p k i q", k=2),
                        func=AF.Exp, scale=float(EXPSC))
                    if b >= 1:
                        emit_tail(b - 1)
                    if b in emit_at:
                        emit_outproj(emit_at[b])
                # outproj(6) needs only cast(9): run it under exp(10)
                emit_outproj(6)
                emit_tail(NBQ - 1)
                emit_outproj(7)
    nc.finalize()
    return nc


def make_in_maps(x, ln_g, ln_b, w_in, b_in, w_out, b_out):
    x = np.asarray(x, np.float32)
    ln_g = np.asarray(ln_g, np.float32)
    ln_b = np.asarray(ln_b, np.float32)
    w_in = np.asarray(w_in, np.float32)
    b_in = np.asarray(b_in, np.float32)
    w_out = np.asarray(w_out, np.float32)
    b_out = np.asarray(b_out, np.float32)

    # fold LN affine into the packed projection (scores descale lives in
    # the exp scale on-device, NOT in the weights)
    w_eff = w_in * ln_g[None, :]
    b_eff = b_in + w_in @ ln_b

    bf = ml_dtypes.bfloat16
    f8 = ml_dtypes.float8_e4m3

    def dr_pack(wT, scale):     # [512, J] -> [128, 2, 2, J] fp8
        J = wT.shape[1]
        return np.ascontiguousarray(
            (wT * scale).reshape(2, 2, 128, J).transpose(2, 0, 1, 3)).astype(f8)

    w8qk = dr_pack(w_eff[:2 * D].T, W8SC)
    w8v = dr_pack(w_eff[2 * D:].T, W8SC)
    wo8 = np.ascontiguousarray(
        (w_out.T * W8SC).reshape(2, 2, 128, D).transpose(0, 2, 1, 3)).astype(f8)
    beffqk = np.ascontiguousarray(
        (b_eff[:2 * D] * W8SC).reshape(8, 128).T).astype(np.float32)
    bo_eff = b_eff[2 * D:] @ w_out.T + b_out      # v-bias folded through Wout

    # DR V slab template: zeros with rowsum ones-columns prebaked
    v8i = np.zeros((128, NBQ, 4, 2, 132), np.float32)
    v8i[:, :, :, 0, 64] = 1.0
    v8i[:, :, :, 1, 130] = 1.0
    v8i = v8i.reshape(128, -1).astype(f8)
    # identity tiled 4x: rhs of the mask-accumulate matmuls
    ie = np.ascontiguousarray(np.tile(np.eye(QB), (1, 4))).astype(bf)

    in_maps = []
    for cidx in range(N_CORES):
        batch = cidx // 4
        t0 = (cidx % 4) * PC
        xloc = np.zeros((STP, D), bf)
        lo = t0 - HALO
        s0, s1 = max(lo, 0), min(t0 + PC + HALO, T)
        xloc[s0 - lo:s1 - lo] = x[batch, s0:s1].astype(bf)
        # partition-major pack: one contiguous descriptor per partition
        xloc = np.ascontiguousarray(
            xloc.reshape(NT, 128, D).transpose(1, 0, 2))
        xr = (x[batch, t0:t0 + PC] + bo_eff[None, :]).astype(np.float32)
        xr = np.ascontiguousarray(xr.reshape(NOB, 128, D).transpose(1, 0, 2))

        # cmask[q, b, w] = MNEG * (1 - mask): band + in-batch bounds
        # (+ block-10 query padding)
        cm = np.zeros((QB, NBQ, 128), np.float32)
        ww = np.arange(128)[None, :]
        qq = np.arange(QB)[:, None]
        for b in range(NBQ):
            w0 = w0_of(b)
            keyg = t0 - HALO + w0 + ww                 # [1, 128]
            qg = t0 + QB * b + qq                      # [QB, 1]
            m = (np.abs(keyg - qg) <= BAND) & (keyg >= 0) & (keyg < T) \
                & (qg < t0 + PC)
            cm[:, b, :] = MNEG * (1.0 - m)
        in_maps.append(dict(
            xin=xloc, xres=xr, w8qk=w8qk, w8v=w8v, wo8=wo8, beffqk=beffqk,
            v8init=v8i, cmask=cm.astype(bf), ieye=ie))
    return in_maps


def kernel_run(inputs, trace=False, trace_kwargs=None):
    global _NC_CACHE
    if _NC_CACHE is None:
        _NC_CACHE = build_bass()
    nc = _NC_CACHE
    in_maps = make_in_maps(**inputs)
    kw = {}
    if trace:
        kw = dict(trace=True, trace_cores=[0], **(trace_kwargs or {}))
    res = run_bass_kernel_spmd(nc, in_maps, list(range(N_CORES)), **kw)
    y = np.stack([res.results[c]["yout"] for c in range(N_CORES)])
    out = y.reshape(B, T, D).astype(np.float32)
    return out, res


def kernel(**inputs):
    out, _ = kernel_run(inputs, trace=False)
    return out


if __name__ == "__main__":
    rng = np.random.default_rng(0)
    ins = dict(
        x=rng.standard_normal((B, T, D)).astype(np.float32),
        ln_g=np.ones(D, np.float32), ln_b=np.zeros(D, np.float32),
        w_in=(rng.standard_normal((3 * D, D)) * 0.02).astype(np.float32),
        b_in=np.zeros(3 * D, np.float32),
        w_out=(rng.standard_normal((D, D)) * 0.02).astype(np.float32),
        b_out=np.zeros(D, np.float32))
    out = kernel(**ins)
    print("ran:", out.shape, out.dtype)
